# revision 45
# baseline (speedup 1.0000x reference)
"""Batched int8 GEMM (s8t x s8n -> s32t) on 8 TRN2 NeuronCores.

out[b, m, n] = sum_k a[b, m, k] * b[b, n, k]   (int32 accumulation)
a: [32, 1024, 1024] int8, b: [32, 1024, 1024] int8 -> out: [32, 1024, 1024] int32

Strategy:
  - Pure batch parallelism: 4 batches per core across 8 cores.
  - Both operands have K innermost, but the PE needs K on partitions.
    DMA-transpose works on 2-byte elements only, so we view the int8
    inputs as uint16 (pairs of adjacent K values) and DMA-transpose
    K-blocks of 256 K-values; each partition holds an even/odd K pair
    interleaved along the free dim. DVE deinterleaves (stride-2 int8
    reads) and converts int8 -> bf16: int8 is exact in bf16; products
    <= 2^14 and sums <= 2^24 are exact in fp32 PSUM accumulation, so
    the GEMM is bit-exact (int8/uint8 matmuls are rejected by the
    verifier, and fp8 decompositions cost >= 2x more PE time, so bf16
    at 1 cycle/row is the fastest exact path).
  - Transpose layout: small per-batch transposes ([1024, 128] u16 ->
    [128, 1024]), all issued on SYNC. Batch 0's first k-block is split
    into halves and the issue order is tuned so each k-block lands
    just before the ramp consumes it (A-h1, needed only by mt group 1
    ~14us later, issues and deints last). Batch bi+1's transposes are
    emitted BEFORE batch bi's stores, so on the SYNC FIFO every DMA's
    completion-semaphore-lane predecessor (the global rotation has
    only 8 lanes across ALL DMAs) is ~8 issues back and long complete.
    Violating this — stores interleaved among pending transposes on
    other engines — serializes both streams (measured 30+us of PE
    starvation + HAM re-throttle).
  - PE: bf16 matmuls, K=128 per instruction, 8-step accumulation into
    [128, 512] fp32 PSUM banks (8 banks in flight; a [128, 1024]
    2-bank PSUM output crashes the backend compiler). 11 dep-free
    dummy matmuls up front warm the HAM clock gate until real data
    lands ~11us in; batch 0 iterates kt-outer over groups of 4 mt
    blocks so the ramp is gapless.
  - ACT copies PSUM fp32 -> SBUF int32 (exact: values are integers).
    After each mt row's two copies, SYNC (done with transpose issues)
    issues a 512KB HWDGE store for that mt row. The 32 stores spread
    across the kernel instead of bunching in the tail (which cost the
    9us tail in the 133us version), and a store stalling on its
    semaphore lane can never block the PSUM-freeing copies (different
    engine). The final mt row uses ACT+DVE copies in parallel and two
    256KB half-stores issued from ACT+SYNC so the kernel tail only
    waits on the last 256KB.
"""

import numpy as np

import concourse.mybir as mybir
import concourse.tile as tile
from concourse import bacc
from concourse.bass_utils import run_bass_kernel_spmd

B, M, N, K = 32, 1024, 1024, 1024
N_CORES = 8
BPC = B // N_CORES  # batches per core
KB = K // 256  # k-blocks of 256 K-values (128 uint16 partitions)
N_TILE = 512
M_TILE = 128

_nc_cache = None


def build_nc():
    nc = bacc.Bacc("TRN2")

    # int8 inputs viewed as uint16 so the xbar DMA-transpose (2-byte
    # granularity) can be used straight out of HBM.
    a_in = nc.dram_tensor("a", [BPC, M, K // 2], mybir.dt.uint16, kind="ExternalInput")
    b_in = nc.dram_tensor("b", [BPC, N, K // 2], mybir.dt.uint16, kind="ExternalInput")
    out = nc.dram_tensor("out", [BPC, M, N], mybir.dt.int32, kind="ExternalOutput")

    n_mt = M // M_TILE
    n_nt = N // N_TILE
    n_kt = 2 * KB

    with tile.TileContext(nc) as tc:
        with (
            tc.tile_pool(name="stage", bufs=1) as stage_pool,
            tc.tile_pool(name="conv", bufs=2) as conv_pool,
            tc.tile_pool(name="psum", bufs=8, space="PSUM") as psum_pool,
            tc.tile_pool(name="outbuf", bufs=8) as out_pool,
            tc.tile_pool(name="warm", bufs=1) as warm_pool,
        ):
            # PE warmup: dummy matmuls with NO deps at all (uninitialized
            # SBUF reads are fine; the PSUM result is discarded), so the
            # HAM clock gate ramps while the first transposes land.
            wsrc = warm_pool.tile([128, N_TILE], mybir.dt.bfloat16, name="wsrc")
            # The tile allocator requires at least one write; 8 columns
            # is enough (the dummies read garbage beyond, which is fine).
            nc.vector.memset(wsrc[:, :8], 0.0)
            wps = psum_pool.tile([128, N_TILE], mybir.dt.float32, name="wps", tag="ps")
            for _ in range(11):
                nc.tensor.matmul(wps[:], wsrc[:, :128], wsrc[:], start=True, stop=True)

            # ---- all transposes up front on SYNC. Batch 0: small
            # per-batch tiles, k-block 0 split in halves (critical-path
            # order A-h0, B-h0, B-h1, A-h1 — the first matmuls need only
            # A-h0 + B-h0). Batches 1-3: 8 big 3-batch transposes. ----
            def stage_small(src, kb, name, m0, m1):
                t = stage_pool.tile(
                    [128, m1 - m0],
                    mybir.dt.uint16,
                    name=f"{name}0_{kb}_{m0}",
                    tag=f"{name}0_{kb}_{m0}",
                )
                nc.sync.dma_start_transpose(t[:], src[0, m0:m1, kb * 128 : (kb + 1) * 128])
                # (int8 view, m0, m1, byte offset of m0 within the tile)
                return (t.bitcast(mybir.dt.int8), m0, m1, 0)

            last_T = [None]  # most recent transpose instruction

            def stage_small_b(src, bi, kb, name):
                t = stage_pool.tile(
                    [128, M],
                    mybir.dt.uint16,
                    name=f"{name}{bi}_{kb}",
                    tag=f"{name}{bi}_{kb}",
                )
                last_T[0] = nc.sync.dma_start_transpose(
                    t[:], src[bi, :, kb * 128 : (kb + 1) * 128]
                )
                return (t.bitcast(mybir.dt.int8), 0, M, 0)

            # Batch 0: first k-block split in halves; A-h1 (only needed
            # by mt group 1, ~14us after the ramp starts) issues and
            # deints last so k-blocks 1-3 arrive just in time for the
            # ramp's consumption.
            a_sm = {0: []}
            b_sm = {0: []}
            a_sm[0].append(stage_small(a_in, 0, "at", 0, M // 2))
            b_sm[0].append(stage_small(b_in, 0, "bt", 0, M // 2))
            b_sm[0].append(stage_small(b_in, 0, "bt", M // 2, M))
            for kb in range(1, KB):
                a_sm[kb] = [stage_small(a_in, kb, "at", 0, M)]
                b_sm[kb] = [stage_small(b_in, kb, "bt", 0, M)]
            a_sm[0].append(stage_small(a_in, 0, "at", M // 2, M))

            # Batches 1-3: small per-batch transposes INTERLEAVED with the
            # previous batch's stores on the SYNC FIFO. The global DMA
            # completion-semaphore rotation has only 8 lanes; emitting
            # 8 transposes then 8 stores as blocks turns adjacent blocks
            # into barriers (a transpose can wait 10us+ on a store 8 slots
            # back — measured, causes PE starvation + HAM re-throttle on
            # jittery runs). Interleaving T,S,T,S keeps every DMA's lane
            # predecessor a recent prompt DMA. Batch bi+1's k-block 0
            # pair is emitted BEFORE bi's stores for extra margin on the
            # critical first deints.
            a_smb = {}
            b_smb = {}

            def stage_one(kind, bi, kb):
                if kind == "A":
                    a_smb.setdefault(bi, {})[kb] = [stage_small_b(a_in, bi, kb, "at")]
                else:
                    b_smb.setdefault(bi, {})[kb] = [stage_small_b(b_in, bi, kb, "bt")]

            # Up-front: ALL of batch 1's and batch 2's transposes, so no
            # store precedes any of them in the DMA semaphore rotation
            # (batch 0's first store is additionally pinned after batch
            # 2's last transpose with an ordering edge — otherwise the
            # scheduler interleaves them and jitter makes batch 2's
            # deints miss their window: measured 2-4us PE gap + HAM
            # re-throttle on ~half the runs). Batch 3's transposes thread
            # through batch 1's stores; their consumer is ~20us away.
            for kb in range(KB):
                stage_one("A", 1, kb)
                stage_one("B", 1, kb)
            stage_one("A", 2, 0)
            stage_one("B", 2, 0)
            pending_by_batch = {
                0: [(k, 2, kb) for kb in (1, 2, 3) for k in ("A", "B")],
                1: [(k, 3, kb) for kb in range(KB) for k in ("A", "B")],
            }
            pending_T = []

            for bi in range(BPC):
                pending_T = pending_by_batch.get(bi, [])
                # ---- deinterleave + int8 -> bf16 (DVE). lhs_ap[kt][mt]
                # and rhs_ap[kt][nt] index into whichever tile holds that
                # m/n range. ----
                lhs_ap = [[None] * n_mt for _ in range(n_kt)]
                rhs_ap = [[None] * n_nt for _ in range(n_kt)]
                deferred_a = []  # batch 0's (kb, A-h1 seg): deints go last
                for kb in range(KB):
                    if bi == 0:
                        a_segs = a_sm[kb]
                        b_segs = b_sm[kb]
                        deferred_a += [(kb, s) for s in a_segs if s[1] != 0]
                        a_segs = [s for s in a_segs if s[1] == 0]
                    else:
                        a_segs = a_smb[bi][kb]
                        b_segs = b_smb[bi][kb]
                    for par in range(2):
                        kt = kb * 2 + par
                        for st8, m0, m1, base in a_segs:
                            abf = conv_pool.tile(
                                [128, m1 - m0],
                                mybir.dt.bfloat16,
                                name=f"abf_{bi}_{kt}_{m0}",
                                tag=f"abf{kt}_{m0}",
                            )
                            nc.vector.tensor_copy(
                                abf[:],
                                st8[:, base + par : base + 2 * (m1 - m0) : 2],
                            )
                            for mt in range(m0 // M_TILE, m1 // M_TILE):
                                o = mt * M_TILE - m0
                                lhs_ap[kt][mt] = abf[:, o : o + M_TILE]
                        for st8, m0, m1, base in b_segs:
                            bbf = conv_pool.tile(
                                [128, m1 - m0],
                                mybir.dt.bfloat16,
                                name=f"bbf_{bi}_{kt}_{m0}",
                                tag=f"bbf{kt}_{m0}",
                            )
                            # All deints on DVE: ACT's ACTIVATE-copy runs
                            # int8 deints ~1.65x slower (measured 1.13us
                            # vs 0.69us full-width) and offloading batch
                            # 0's B-deints to it perturbed the transpose
                            # schedule enough to re-open ramp gaps.
                            nc.vector.tensor_copy(
                                bbf[:],
                                st8[:, base + par : base + 2 * (m1 - m0) : 2],
                            )
                            for nt in range(m0 // N_TILE, (m1 + N_TILE - 1) // N_TILE):
                                o = nt * N_TILE - m0
                                rhs_ap[kt][nt] = bbf[:, o : o + N_TILE]

                # Batch 0's A-h1 deints last on the DVE FIFO: that data
                # (lhs for mt 4-7) is only needed at mt group 1 (~14us
                # after the ramp starts), and its transposes issue late —
                # emitting them in k-block order would head-of-line block
                # the ramp-critical deints.
                for kb, (st8, m0, m1, base) in deferred_a:
                    for par in range(2):
                        kt = kb * 2 + par
                        abf = conv_pool.tile(
                            [128, m1 - m0],
                            mybir.dt.bfloat16,
                            name=f"abf_{bi}_{kt}_{m0}",
                            tag=f"abf{kt}_{m0}",
                        )
                        nc.vector.tensor_copy(
                            abf[:],
                            st8[:, base + par : base + 2 * (m1 - m0) : 2],
                        )
                        for mt in range(m0 // M_TILE, m1 // M_TILE):
                            o = mt * M_TILE - m0
                            lhs_ap[kt][mt] = abf[:, o : o + M_TILE]

                # ---- GEMM, accumulating in PSUM over kt. After each mt
                # row's two PSUM-freeing copies (ACT), SYNC issues that
                # row's 512KB store. ----
                def emit_store(mt, ot):
                    st = nc.sync.dma_start(
                        out[bi, mt * M_TILE : (mt + 1) * M_TILE, :], ot[:]
                    )
                    if pending_T:
                        stage_one(*pending_T.pop(0))

                if bi == 0:
                    # Batch 0 is rate-limited by the transpose + deint
                    # stream: iterate kt-outer over groups of 4 mt blocks
                    # (8 PSUM banks) so each arriving k-tile feeds 1.7us
                    # of real PE work and the ramp is gapless. nt-outer
                    # inside each kt so the first 4 matmuls only need
                    # B-half0.
                    for g in range(n_mt // 4):
                        mts = range(4 * g, 4 * g + 4)
                        ps = {
                            (mt, nt): psum_pool.tile(
                                [128, N_TILE],
                                mybir.dt.float32,
                                name=f"ps_{bi}_{mt}_{nt}",
                                tag="ps",
                            )
                            for mt in mts
                            for nt in range(n_nt)
                        }
                        for kt in range(n_kt):
                            for nt in range(n_nt):
                                for mt in mts:
                                    nc.tensor.matmul(
                                        ps[(mt, nt)][:],
                                        lhs_ap[kt][mt],
                                        rhs_ap[kt][nt],
                                        start=(kt == 0),
                                        stop=(kt == n_kt - 1),
                                    )
                        for mt in mts:
                            ot = out_pool.tile(
                                [128, N], mybir.dt.int32, name=f"ot_{bi}_{mt}", tag="ot"
                            )
                            for nt in range(n_nt):
                                nc.scalar.copy(
                                    ot[:, nt * N_TILE : (nt + 1) * N_TILE],
                                    ps[(mt, nt)][:],
                                )
                            emit_store(mt, ot)
                else:
                    # Steady-state batches: mt-outer so the PSUM-freeing
                    # copies and stores spread evenly.
                    for mt in range(n_mt):
                        ps = [
                            psum_pool.tile(
                                [128, N_TILE],
                                mybir.dt.float32,
                                name=f"ps_{bi}_{mt}_{nt}",
                                tag="ps",
                            )
                            for nt in range(n_nt)
                        ]
                        for kt in range(n_kt):
                            for nt in range(n_nt):
                                nc.tensor.matmul(
                                    ps[nt][:],
                                    lhs_ap[kt][mt],
                                    rhs_ap[kt][nt],
                                    start=(kt == 0),
                                    stop=(kt == n_kt - 1),
                                )
                        if bi == BPC - 1 and mt == n_mt - 1:
                            # Final mt row: ACT+DVE copies in parallel into
                            # two half tiles, two 256KB half-stores, so the
                            # kernel tail only waits on the last 256KB.
                            o0 = out_pool.tile(
                                [128, N_TILE],
                                mybir.dt.int32,
                                name="ot_l0",
                                tag="otl0",
                                bufs=1,
                            )
                            o1 = out_pool.tile(
                                [128, N_TILE],
                                mybir.dt.int32,
                                name="ot_l1",
                                tag="otl1",
                                bufs=1,
                            )
                            # Issue the two half-stores from different
                            # engines (ACT + SYNC) so they go out in
                            # parallel; by now no transposes are pending,
                            # so the ACT-ring store cannot stall copies.
                            nc.scalar.copy(o0[:], ps[0][:])
                            nc.vector.tensor_copy(o1[:], ps[1][:])
                            nc.scalar.dma_start(
                                out[bi, mt * M_TILE : (mt + 1) * M_TILE, :N_TILE],
                                o0[:],
                            )
                            nc.sync.dma_start(
                                out[bi, mt * M_TILE : (mt + 1) * M_TILE, N_TILE:],
                                o1[:],
                            )
                        else:
                            ot = out_pool.tile(
                                [128, N], mybir.dt.int32, name=f"ot_{bi}_{mt}", tag="ot"
                            )
                            for nt in range(n_nt):
                                nc.scalar.copy(
                                    ot[:, nt * N_TILE : (nt + 1) * N_TILE], ps[nt][:]
                                )
                            emit_store(mt, ot)
    nc.compile()
    return nc


def _get_nc():
    global _nc_cache
    if _nc_cache is None:
        _nc_cache = build_nc()
    return _nc_cache


def run(a: np.ndarray, b: np.ndarray, trace: bool = False):
    """Run on 8 cores. a/b: [32, 1024, 1024] int8. Returns (out, BassKernelResults)."""
    a = np.ascontiguousarray(a)
    b = np.ascontiguousarray(b)
    a16 = a.view(np.uint16).reshape(B, M, K // 2)
    b16 = b.view(np.uint16).reshape(B, N, K // 2)
    in_maps = [
        {
            "a": a16[c * BPC : (c + 1) * BPC],
            "b": b16[c * BPC : (c + 1) * BPC],
        }
        for c in range(N_CORES)
    ]
    res = run_bass_kernel_spmd(_get_nc(), in_maps, list(range(N_CORES)), trace=trace)
    out = np.concatenate([res.results[c]["out"] for c in range(N_CORES)], axis=0)
    return out, res


def kernel(a: np.ndarray, b: np.ndarray) -> np.ndarray:
    out, _ = run(np.asarray(a), np.asarray(b))
    return out


# revision 46
# speedup vs baseline: 1.0859x; 1.0859x over previous
"""Batched int8 GEMM (s8t x s8n -> s32t) on 8 TRN2 NeuronCores.

out[b, m, n] = sum_k a[b, m, k] * b[b, n, k]   (int32 accumulation)
a: [32, 1024, 1024] int8, b: [32, 1024, 1024] int8 -> out: [32, 1024, 1024] int32

Strategy:
  - Pure batch parallelism: 4 batches per core across 8 cores.
  - Both operands have K innermost, but the PE needs K on partitions.
    DMA-transpose works on 2-byte elements only, so we view the int8
    inputs as uint16 (pairs of adjacent K values) and DMA-transpose
    K-blocks of 256 K-values; each partition holds an even/odd K pair
    interleaved along the free dim. DVE deinterleaves (stride-2 int8
    reads) and converts int8 -> bf16: int8 is exact in bf16; products
    <= 2^14 and sums <= 2^24 are exact in fp32 PSUM accumulation, so
    the GEMM is bit-exact (int8/uint8 matmuls are rejected by the
    verifier, and fp8 decompositions cost >= 2x more PE time, so bf16
    at 1 cycle/row is the fastest exact path).
  - Transpose layout: small per-batch transposes ([1024, 128] u16 ->
    [128, 1024]), all issued on SYNC. Batch 0's first k-block is split
    into halves and the issue order is tuned so each k-block lands
    just before the ramp consumes it (A-h1, needed only by mt group 1
    ~14us later, issues and deints last). Batch bi+1's transposes are
    emitted BEFORE batch bi's stores, so on the SYNC FIFO every DMA's
    completion-semaphore-lane predecessor (the global rotation has
    only 8 lanes across ALL DMAs) is ~8 issues back and long complete.
    Violating this — stores interleaved among pending transposes on
    other engines — serializes both streams (measured 30+us of PE
    starvation + HAM re-throttle).
  - PE: bf16 matmuls, K=128 per instruction, 8-step accumulation into
    [128, 512] fp32 PSUM banks (8 banks in flight; a [128, 1024]
    2-bank PSUM output crashes the backend compiler). 11 dep-free
    dummy matmuls up front warm the HAM clock gate until real data
    lands ~11us in; batch 0 iterates kt-outer over groups of 4 mt
    blocks so the ramp is gapless.
  - ACT copies PSUM fp32 -> SBUF int32 (exact: values are integers).
    After each mt row's two copies, SYNC (done with transpose issues)
    issues a 512KB HWDGE store for that mt row. The 32 stores spread
    across the kernel instead of bunching in the tail (which cost the
    9us tail in the 133us version), and a store stalling on its
    semaphore lane can never block the PSUM-freeing copies (different
    engine). The final mt row uses ACT+DVE copies in parallel and two
    256KB half-stores issued from ACT+SYNC so the kernel tail only
    waits on the last 256KB.
"""

import numpy as np

import concourse.mybir as mybir
import concourse.tile as tile
from concourse import bacc
from concourse.bass_utils import run_bass_kernel_spmd

B, M, N, K = 32, 1024, 1024, 1024
N_CORES = 8
BPC = B // N_CORES  # batches per core
KB = K // 256  # k-blocks of 256 K-values (128 uint16 partitions)
N_TILE = 512
M_TILE = 128

_nc_cache = None


def build_nc():
    nc = bacc.Bacc("TRN2")

    # int8 inputs viewed as uint16 so the xbar DMA-transpose (2-byte
    # granularity) can be used straight out of HBM.
    a_in = nc.dram_tensor("a", [BPC, M, K // 2], mybir.dt.uint16, kind="ExternalInput")
    b_in = nc.dram_tensor("b", [BPC, N, K // 2], mybir.dt.uint16, kind="ExternalInput")
    out = nc.dram_tensor("out", [BPC, M, N], mybir.dt.int32, kind="ExternalOutput")

    n_mt = M // M_TILE
    n_nt = N // N_TILE
    n_kt = 2 * KB

    with tile.TileContext(nc) as tc:
        with (
            tc.tile_pool(name="stage", bufs=1) as stage_pool,
            tc.tile_pool(name="conv", bufs=2) as conv_pool,
            tc.tile_pool(name="psum", bufs=8, space="PSUM") as psum_pool,
            tc.tile_pool(name="outbuf", bufs=8) as out_pool,
            tc.tile_pool(name="warm", bufs=1) as warm_pool,
        ):
            # PE warmup: dummy matmuls with NO deps at all (uninitialized
            # SBUF reads are fine; the PSUM result is discarded), so the
            # HAM clock gate ramps while the first transposes land.
            wsrc = warm_pool.tile([128, N_TILE], mybir.dt.bfloat16, name="wsrc")
            # The tile allocator requires at least one write; 8 columns
            # is enough (the dummies read garbage beyond, which is fine).
            nc.vector.memset(wsrc[:, :8], 0.0)
            wps = psum_pool.tile([128, N_TILE], mybir.dt.float32, name="wps", tag="ps")
            for _ in range(11):
                nc.tensor.matmul(wps[:], wsrc[:, :128], wsrc[:], start=True, stop=True)

            # ---- all transposes up front on SYNC. Batch 0: small
            # per-batch tiles, k-block 0 split in halves (critical-path
            # order A-h0, B-h0, B-h1, A-h1 — the first matmuls need only
            # A-h0 + B-h0). Batches 1-3: 8 big 3-batch transposes. ----
            def stage_small(src, kb, name, m0, m1):
                t = stage_pool.tile(
                    [128, m1 - m0],
                    mybir.dt.uint16,
                    name=f"{name}0_{kb}_{m0}",
                    tag=f"{name}0_{kb}_{m0}",
                )
                nc.sync.dma_start_transpose(t[:], src[0, m0:m1, kb * 128 : (kb + 1) * 128])
                # (int8 view, m0, m1, byte offset of m0 within the tile)
                return (t.bitcast(mybir.dt.int8), m0, m1, 0)

            last_T = [None]  # most recent transpose instruction

            def stage_small_b(src, bi, kb, name):
                t = stage_pool.tile(
                    [128, M],
                    mybir.dt.uint16,
                    name=f"{name}{bi}_{kb}",
                    tag=f"{name}{bi}_{kb}",
                )
                last_T[0] = nc.sync.dma_start_transpose(
                    t[:], src[bi, :, kb * 128 : (kb + 1) * 128]
                )
                return (t.bitcast(mybir.dt.int8), 0, M, 0)

            # Batch 0: first k-block split in halves; A-h1 (only needed
            # by mt group 1, ~14us after the ramp starts) issues and
            # deints last so k-blocks 1-3 arrive just in time for the
            # ramp's consumption.
            a_sm = {0: []}
            b_sm = {0: []}
            a_sm[0].append(stage_small(a_in, 0, "at", 0, M // 2))
            b_sm[0].append(stage_small(b_in, 0, "bt", 0, M // 2))
            b_sm[0].append(stage_small(b_in, 0, "bt", M // 2, M))
            for kb in range(1, KB):
                a_sm[kb] = [stage_small(a_in, kb, "at", 0, M)]
                b_sm[kb] = [stage_small(b_in, kb, "bt", 0, M)]
            a_sm[0].append(stage_small(a_in, 0, "at", M // 2, M))

            # Batches 1-3: small per-batch transposes INTERLEAVED with the
            # previous batch's stores on the SYNC FIFO. The global DMA
            # completion-semaphore rotation has only 8 lanes; emitting
            # 8 transposes then 8 stores as blocks turns adjacent blocks
            # into barriers (a transpose can wait 10us+ on a store 8 slots
            # back — measured, causes PE starvation + HAM re-throttle on
            # jittery runs). Interleaving T,S,T,S keeps every DMA's lane
            # predecessor a recent prompt DMA. Batch bi+1's k-block 0
            # pair is emitted BEFORE bi's stores for extra margin on the
            # critical first deints.
            a_smb = {}
            b_smb = {}

            def stage_one(kind, bi, kb):
                if kind == "A":
                    a_smb.setdefault(bi, {})[kb] = [stage_small_b(a_in, bi, kb, "at")]
                else:
                    b_smb.setdefault(bi, {})[kb] = [stage_small_b(b_in, bi, kb, "bt")]

            # Up-front: ALL of batch 1's and batch 2's transposes, so no
            # store precedes any of them in the DMA semaphore rotation
            # (batch 0's first store is additionally pinned after batch
            # 2's last transpose with an ordering edge — otherwise the
            # scheduler interleaves them and jitter makes batch 2's
            # deints miss their window: measured 2-4us PE gap + HAM
            # re-throttle on ~half the runs). Batch 3's transposes thread
            # through batch 1's stores; their consumer is ~20us away.
            for kb in range(KB):
                stage_one("A", 1, kb)
                stage_one("B", 1, kb)
            stage_one("A", 2, 0)
            stage_one("B", 2, 0)
            pending_by_batch = {
                0: [(k, 2, kb) for kb in (1, 2, 3) for k in ("A", "B")],
                1: [(k, 3, kb) for kb in range(KB) for k in ("A", "B")],
            }
            pending_T = []

            for bi in range(BPC):
                pending_T = pending_by_batch.get(bi, [])
                # ---- deinterleave + int8 -> bf16 (DVE). lhs_ap[kt][mt]
                # and rhs_ap[kt][nt] index into whichever tile holds that
                # m/n range. ----
                lhs_ap = [[None] * n_mt for _ in range(n_kt)]
                rhs_ap = [[None] * n_nt for _ in range(n_kt)]
                deferred_a = []  # batch 0's (kb, A-h1 seg): deints go last
                for kb in range(KB):
                    if bi == 0:
                        # Batch 0 deints in HALVES (sliced from the full
                        # staged tiles for kb>=1 — transposes unchanged):
                        # mt group 0 never reads A's upper half, so those
                        # deints defer to the idle stretch after kb3, and
                        # B's h0 unlocks each kt's nt0 matmuls ~0.7us
                        # sooner. This halves the serial DVE critical
                        # path that caused a ~1.1us ramp gap.
                        if kb == 0:
                            a_segs = a_sm[kb]
                            b_segs = b_sm[kb]
                        else:
                            at8 = a_sm[kb][0][0]
                            bt8 = b_sm[kb][0][0]
                            a_segs = [(at8, 0, M // 2, 0), (at8, M // 2, M, M)]
                            b_segs = [(bt8, 0, M // 2, 0), (bt8, M // 2, M, M)]
                        deferred_a += [(kb, s) for s in a_segs if s[1] != 0]
                        a_segs = [s for s in a_segs if s[1] == 0]
                    else:
                        a_segs = a_smb[bi][kb]
                        b_segs = b_smb[bi][kb]
                    for par in range(2):
                        kt = kb * 2 + par
                        for st8, m0, m1, base in a_segs:
                            abf = conv_pool.tile(
                                [128, m1 - m0],
                                mybir.dt.bfloat16,
                                name=f"abf_{bi}_{kt}_{m0}",
                                tag=f"abf{kt}_{m0}",
                            )
                            nc.vector.tensor_copy(
                                abf[:],
                                st8[:, base + par : base + 2 * (m1 - m0) : 2],
                            )
                            for mt in range(m0 // M_TILE, m1 // M_TILE):
                                o = mt * M_TILE - m0
                                lhs_ap[kt][mt] = abf[:, o : o + M_TILE]
                        for st8, m0, m1, base in b_segs:
                            bbf = conv_pool.tile(
                                [128, m1 - m0],
                                mybir.dt.bfloat16,
                                name=f"bbf_{bi}_{kt}_{m0}",
                                tag=f"bbf{kt}_{m0}",
                            )
                            # All deints on DVE: ACT's ACTIVATE-copy runs
                            # int8 deints ~1.65x slower (measured 1.13us
                            # vs 0.69us full-width) and offloading batch
                            # 0's B-deints to it perturbed the transpose
                            # schedule enough to re-open ramp gaps.
                            nc.vector.tensor_copy(
                                bbf[:],
                                st8[:, base + par : base + 2 * (m1 - m0) : 2],
                            )
                            for nt in range(m0 // N_TILE, (m1 + N_TILE - 1) // N_TILE):
                                o = nt * N_TILE - m0
                                rhs_ap[kt][nt] = bbf[:, o : o + N_TILE]

                # Batch 0's A-h1 deints last on the DVE FIFO: that data
                # (lhs for mt 4-7) is only needed at mt group 1 (~14us
                # after the ramp starts), and its transposes issue late —
                # emitting them in k-block order would head-of-line block
                # the ramp-critical deints.
                for kb, (st8, m0, m1, base) in deferred_a:
                    for par in range(2):
                        kt = kb * 2 + par
                        abf = conv_pool.tile(
                            [128, m1 - m0],
                            mybir.dt.bfloat16,
                            name=f"abf_{bi}_{kt}_{m0}",
                            tag=f"abf{kt}_{m0}",
                        )
                        nc.vector.tensor_copy(
                            abf[:],
                            st8[:, base + par : base + 2 * (m1 - m0) : 2],
                        )
                        for mt in range(m0 // M_TILE, m1 // M_TILE):
                            o = mt * M_TILE - m0
                            lhs_ap[kt][mt] = abf[:, o : o + M_TILE]

                # ---- GEMM, accumulating in PSUM over kt. After each mt
                # row's two PSUM-freeing copies (ACT), SYNC issues that
                # row's 512KB store. ----
                def emit_store(mt, ot):
                    st = nc.sync.dma_start(
                        out[bi, mt * M_TILE : (mt + 1) * M_TILE, :], ot[:]
                    )
                    if pending_T:
                        stage_one(*pending_T.pop(0))

                if bi == 0:
                    # Batch 0 is rate-limited by the transpose + deint
                    # stream: iterate kt-outer over groups of 4 mt blocks
                    # (8 PSUM banks) so each arriving k-tile feeds 1.7us
                    # of real PE work and the ramp is gapless. nt-outer
                    # inside each kt so the first 4 matmuls only need
                    # B-half0.
                    for g in range(n_mt // 4):
                        mts = range(4 * g, 4 * g + 4)
                        ps = {
                            (mt, nt): psum_pool.tile(
                                [128, N_TILE],
                                mybir.dt.float32,
                                name=f"ps_{bi}_{mt}_{nt}",
                                tag="ps",
                            )
                            for mt in mts
                            for nt in range(n_nt)
                        }
                        for kt in range(n_kt):
                            for nt in range(n_nt):
                                for mt in mts:
                                    nc.tensor.matmul(
                                        ps[(mt, nt)][:],
                                        lhs_ap[kt][mt],
                                        rhs_ap[kt][nt],
                                        start=(kt == 0),
                                        stop=(kt == n_kt - 1),
                                    )
                        for mt in mts:
                            ot = out_pool.tile(
                                [128, N], mybir.dt.int32, name=f"ot_{bi}_{mt}", tag="ot"
                            )
                            for nt in range(n_nt):
                                nc.scalar.copy(
                                    ot[:, nt * N_TILE : (nt + 1) * N_TILE],
                                    ps[(mt, nt)][:],
                                )
                            emit_store(mt, ot)
                else:
                    # Steady-state batches: mt-outer so the PSUM-freeing
                    # copies and stores spread evenly.
                    for mt in range(n_mt):
                        ps = [
                            psum_pool.tile(
                                [128, N_TILE],
                                mybir.dt.float32,
                                name=f"ps_{bi}_{mt}_{nt}",
                                tag="ps",
                            )
                            for nt in range(n_nt)
                        ]
                        for kt in range(n_kt):
                            for nt in range(n_nt):
                                nc.tensor.matmul(
                                    ps[nt][:],
                                    lhs_ap[kt][mt],
                                    rhs_ap[kt][nt],
                                    start=(kt == 0),
                                    stop=(kt == n_kt - 1),
                                )
                        if bi == BPC - 1 and mt == n_mt - 1:
                            # Final mt row: ACT+DVE copies in parallel into
                            # two half tiles, two 256KB half-stores, so the
                            # kernel tail only waits on the last 256KB.
                            o0 = out_pool.tile(
                                [128, N_TILE],
                                mybir.dt.int32,
                                name="ot_l0",
                                tag="otl0",
                                bufs=1,
                            )
                            o1 = out_pool.tile(
                                [128, N_TILE],
                                mybir.dt.int32,
                                name="ot_l1",
                                tag="otl1",
                                bufs=1,
                            )
                            # Issue the two half-stores from different
                            # engines (ACT + SYNC) so they go out in
                            # parallel; by now no transposes are pending,
                            # so the ACT-ring store cannot stall copies.
                            nc.scalar.copy(o0[:], ps[0][:])
                            nc.vector.tensor_copy(o1[:], ps[1][:])
                            nc.scalar.dma_start(
                                out[bi, mt * M_TILE : (mt + 1) * M_TILE, :N_TILE],
                                o0[:],
                            )
                            nc.sync.dma_start(
                                out[bi, mt * M_TILE : (mt + 1) * M_TILE, N_TILE:],
                                o1[:],
                            )
                        else:
                            ot = out_pool.tile(
                                [128, N], mybir.dt.int32, name=f"ot_{bi}_{mt}", tag="ot"
                            )
                            for nt in range(n_nt):
                                nc.scalar.copy(
                                    ot[:, nt * N_TILE : (nt + 1) * N_TILE], ps[nt][:]
                                )
                            emit_store(mt, ot)
    nc.compile()
    return nc


def _get_nc():
    global _nc_cache
    if _nc_cache is None:
        _nc_cache = build_nc()
    return _nc_cache


def run(a: np.ndarray, b: np.ndarray, trace: bool = False):
    """Run on 8 cores. a/b: [32, 1024, 1024] int8. Returns (out, BassKernelResults)."""
    a = np.ascontiguousarray(a)
    b = np.ascontiguousarray(b)
    a16 = a.view(np.uint16).reshape(B, M, K // 2)
    b16 = b.view(np.uint16).reshape(B, N, K // 2)
    in_maps = [
        {
            "a": a16[c * BPC : (c + 1) * BPC],
            "b": b16[c * BPC : (c + 1) * BPC],
        }
        for c in range(N_CORES)
    ]
    res = run_bass_kernel_spmd(_get_nc(), in_maps, list(range(N_CORES)), trace=trace)
    out = np.concatenate([res.results[c]["out"] for c in range(N_CORES)], axis=0)
    return out, res


def kernel(a: np.ndarray, b: np.ndarray) -> np.ndarray:
    out, _ = run(np.asarray(a), np.asarray(b))
    return out


# revision 47
# speedup vs baseline: 1.1846x; 1.0909x over previous
"""Batched int8 GEMM (s8t x s8n -> s32t) on 8 TRN2 NeuronCores.

out[b, m, n] = sum_k a[b, m, k] * b[b, n, k]   (int32 accumulation)
a: [32, 1024, 1024] int8, b: [32, 1024, 1024] int8 -> out: [32, 1024, 1024] int32

Strategy:
  - Pure batch parallelism: 4 batches per core across 8 cores.
  - Both operands have K innermost, but the PE needs K on partitions.
    DMA-transpose works on 2-byte elements only, so we view the int8
    inputs as uint16 (pairs of adjacent K values) and DMA-transpose
    K-blocks of 256 K-values; each partition holds an even/odd K pair
    interleaved along the free dim. DVE deinterleaves (stride-2 int8
    reads) and converts int8 -> bf16: int8 is exact in bf16; products
    <= 2^14 and sums <= 2^24 are exact in fp32 PSUM accumulation, so
    the GEMM is bit-exact (int8/uint8 matmuls are rejected by the
    verifier, and fp8 decompositions cost >= 2x more PE time, so bf16
    at 1 cycle/row is the fastest exact path).
  - Transpose layout: small per-batch transposes ([1024, 128] u16 ->
    [128, 1024]), all issued on SYNC. Batch 0's first k-block is split
    into halves and the issue order is tuned so each k-block lands
    just before the ramp consumes it (A-h1, needed only by mt group 1
    ~14us later, issues and deints last). Batch bi+1's transposes are
    emitted BEFORE batch bi's stores, so on the SYNC FIFO every DMA's
    completion-semaphore-lane predecessor (the global rotation has
    only 8 lanes across ALL DMAs) is ~8 issues back and long complete.
    Violating this — stores interleaved among pending transposes on
    other engines — serializes both streams (measured 30+us of PE
    starvation + HAM re-throttle).
  - PE: bf16 matmuls, K=128 per instruction, 8-step accumulation into
    [128, 512] fp32 PSUM banks (8 banks in flight; a [128, 1024]
    2-bank PSUM output crashes the backend compiler). 12 dep-free
    dummy matmuls up front warm the HAM clock gate until real data
    lands ~11us in; batch 0 iterates kt-outer over groups of 4 mt
    blocks so the ramp is gapless.
  - ACT copies PSUM fp32 -> SBUF int32 (exact: values are integers).
    After each mt row's two copies, SYNC (done with transpose issues)
    issues a 512KB HWDGE store for that mt row. The 32 stores spread
    across the kernel instead of bunching in the tail (which cost the
    9us tail in the 133us version), and a store stalling on its
    semaphore lane can never block the PSUM-freeing copies (different
    engine). The final mt row uses ACT+DVE copies in parallel and two
    256KB half-stores issued from ACT+SYNC so the kernel tail only
    waits on the last 256KB.
"""

import numpy as np

import concourse.mybir as mybir
import concourse.tile as tile
from concourse import bacc
from concourse.bass_utils import run_bass_kernel_spmd

B, M, N, K = 32, 1024, 1024, 1024
N_CORES = 8
BPC = B // N_CORES  # batches per core
KB = K // 256  # k-blocks of 256 K-values (128 uint16 partitions)
N_TILE = 512
M_TILE = 128

_nc_cache = None


def build_nc():
    nc = bacc.Bacc("TRN2")

    # int8 inputs viewed as uint16 so the xbar DMA-transpose (2-byte
    # granularity) can be used straight out of HBM.
    a_in = nc.dram_tensor("a", [BPC, M, K // 2], mybir.dt.uint16, kind="ExternalInput")
    b_in = nc.dram_tensor("b", [BPC, N, K // 2], mybir.dt.uint16, kind="ExternalInput")
    out = nc.dram_tensor("out", [BPC, M, N], mybir.dt.int32, kind="ExternalOutput")

    n_mt = M // M_TILE
    n_nt = N // N_TILE
    n_kt = 2 * KB

    with tile.TileContext(nc) as tc:
        with (
            tc.tile_pool(name="stage", bufs=1) as stage_pool,
            tc.tile_pool(name="conv", bufs=2) as conv_pool,
            tc.tile_pool(name="psum", bufs=8, space="PSUM") as psum_pool,
            tc.tile_pool(name="outbuf", bufs=8) as out_pool,
            tc.tile_pool(name="warm", bufs=1) as warm_pool,
        ):
            # PE warmup: dummy matmuls with NO deps at all (uninitialized
            # SBUF reads are fine; the PSUM result is discarded), so the
            # HAM clock gate ramps while the first transposes land.
            wsrc = warm_pool.tile([128, N_TILE], mybir.dt.bfloat16, name="wsrc")
            # The tile allocator requires at least one write; 8 columns
            # is enough (the dummies read garbage beyond, which is fine).
            nc.vector.memset(wsrc[:, :8], 0.0)
            wps = psum_pool.tile([128, N_TILE], mybir.dt.float32, name="wps", tag="ps")
            for _ in range(12):
                nc.tensor.matmul(wps[:], wsrc[:, :128], wsrc[:], start=True, stop=True)

            # ---- all transposes up front on SYNC. Batch 0: small
            # per-batch tiles, k-block 0 split in halves (critical-path
            # order A-h0, B-h0, B-h1, A-h1 — the first matmuls need only
            # A-h0 + B-h0). Batches 1-3: 8 big 3-batch transposes. ----
            def stage_small(src, kb, name, m0, m1):
                t = stage_pool.tile(
                    [128, m1 - m0],
                    mybir.dt.uint16,
                    name=f"{name}0_{kb}_{m0}",
                    tag=f"{name}0_{kb}_{m0}",
                )
                nc.sync.dma_start_transpose(t[:], src[0, m0:m1, kb * 128 : (kb + 1) * 128])
                # (int8 view, m0, m1, byte offset of m0 within the tile)
                return (t.bitcast(mybir.dt.int8), m0, m1, 0)

            last_T = [None]  # most recent transpose instruction

            def stage_small_b(src, bi, kb, name):
                t = stage_pool.tile(
                    [128, M],
                    mybir.dt.uint16,
                    name=f"{name}{bi}_{kb}",
                    tag=f"{name}{bi}_{kb}",
                )
                last_T[0] = nc.sync.dma_start_transpose(
                    t[:], src[bi, :, kb * 128 : (kb + 1) * 128]
                )
                return (t.bitcast(mybir.dt.int8), 0, M, 0)

            # Batch 0: first k-block split in halves; A-h1 (only needed
            # by mt group 1, ~14us after the ramp starts) issues and
            # deints last so k-blocks 1-3 arrive just in time for the
            # ramp's consumption.
            a_sm = {0: []}
            b_sm = {0: []}
            a_sm[0].append(stage_small(a_in, 0, "at", 0, M // 2))
            b_sm[0].append(stage_small(b_in, 0, "bt", 0, M // 2))
            b_sm[0].append(stage_small(b_in, 0, "bt", M // 2, M))
            for kb in range(1, KB):
                a_sm[kb] = [stage_small(a_in, kb, "at", 0, M)]
                b_sm[kb] = [stage_small(b_in, kb, "bt", 0, M)]
            a_sm[0].append(stage_small(a_in, 0, "at", M // 2, M))

            # Batches 1-3: small per-batch transposes INTERLEAVED with the
            # previous batch's stores on the SYNC FIFO. The global DMA
            # completion-semaphore rotation has only 8 lanes; emitting
            # 8 transposes then 8 stores as blocks turns adjacent blocks
            # into barriers (a transpose can wait 10us+ on a store 8 slots
            # back — measured, causes PE starvation + HAM re-throttle on
            # jittery runs). Interleaving T,S,T,S keeps every DMA's lane
            # predecessor a recent prompt DMA. Batch bi+1's k-block 0
            # pair is emitted BEFORE bi's stores for extra margin on the
            # critical first deints.
            a_smb = {}
            b_smb = {}

            def stage_one(kind, bi, kb):
                if kind == "A":
                    a_smb.setdefault(bi, {})[kb] = [stage_small_b(a_in, bi, kb, "at")]
                else:
                    b_smb.setdefault(bi, {})[kb] = [stage_small_b(b_in, bi, kb, "bt")]

            # Up-front: ALL of batch 1's and batch 2's transposes, so no
            # store precedes any of them in the DMA semaphore rotation
            # (batch 0's first store is additionally pinned after batch
            # 2's last transpose with an ordering edge — otherwise the
            # scheduler interleaves them and jitter makes batch 2's
            # deints miss their window: measured 2-4us PE gap + HAM
            # re-throttle on ~half the runs). Batch 3's transposes thread
            # through batch 1's stores; their consumer is ~20us away.
            for kb in range(KB):
                stage_one("A", 1, kb)
                stage_one("B", 1, kb)
            stage_one("A", 2, 0)
            stage_one("B", 2, 0)
            pending_by_batch = {
                0: [(k, 2, kb) for kb in (1, 2, 3) for k in ("A", "B")],
                1: [(k, 3, kb) for kb in range(KB) for k in ("A", "B")],
            }
            pending_T = []

            for bi in range(BPC):
                pending_T = pending_by_batch.get(bi, [])
                # ---- deinterleave + int8 -> bf16 (DVE). lhs_ap[kt][mt]
                # and rhs_ap[kt][nt] index into whichever tile holds that
                # m/n range. ----
                lhs_ap = [[None] * n_mt for _ in range(n_kt)]
                rhs_ap = [[None] * n_nt for _ in range(n_kt)]
                deferred_a = []  # batch 0's (kb, A-h1 seg): deints go last
                for kb in range(KB):
                    if bi == 0:
                        # Batch 0 deints in HALVES (sliced from the full
                        # staged tiles for kb>=1 — transposes unchanged):
                        # mt group 0 never reads A's upper half, so those
                        # deints defer to the idle stretch after kb3, and
                        # B's h0 unlocks each kt's nt0 matmuls ~0.7us
                        # sooner. This halves the serial DVE critical
                        # path that caused a ~1.1us ramp gap.
                        if kb == 0:
                            a_segs = a_sm[kb]
                            b_segs = b_sm[kb]
                        else:
                            at8 = a_sm[kb][0][0]
                            bt8 = b_sm[kb][0][0]
                            a_segs = [(at8, 0, M // 2, 0), (at8, M // 2, M, M)]
                            b_segs = [(bt8, 0, M // 2, 0), (bt8, M // 2, M, M)]
                        deferred_a += [(kb, s) for s in a_segs if s[1] != 0]
                        a_segs = [s for s in a_segs if s[1] == 0]
                    else:
                        a_segs = a_smb[bi][kb]
                        b_segs = b_smb[bi][kb]
                    for par in range(2):
                        kt = kb * 2 + par
                        for st8, m0, m1, base in a_segs:
                            abf = conv_pool.tile(
                                [128, m1 - m0],
                                mybir.dt.bfloat16,
                                name=f"abf_{bi}_{kt}_{m0}",
                                tag=f"abf{kt}_{m0}",
                            )
                            nc.vector.tensor_copy(
                                abf[:],
                                st8[:, base + par : base + 2 * (m1 - m0) : 2],
                            )
                            for mt in range(m0 // M_TILE, m1 // M_TILE):
                                o = mt * M_TILE - m0
                                lhs_ap[kt][mt] = abf[:, o : o + M_TILE]
                        for st8, m0, m1, base in b_segs:
                            bbf = conv_pool.tile(
                                [128, m1 - m0],
                                mybir.dt.bfloat16,
                                name=f"bbf_{bi}_{kt}_{m0}",
                                tag=f"bbf{kt}_{m0}",
                            )
                            # All deints on DVE: ACT's ACTIVATE-copy runs
                            # int8 deints ~1.65x slower (measured 1.13us
                            # vs 0.69us full-width) and offloading batch
                            # 0's B-deints to it perturbed the transpose
                            # schedule enough to re-open ramp gaps.
                            nc.vector.tensor_copy(
                                bbf[:],
                                st8[:, base + par : base + 2 * (m1 - m0) : 2],
                            )
                            for nt in range(m0 // N_TILE, (m1 + N_TILE - 1) // N_TILE):
                                o = nt * N_TILE - m0
                                rhs_ap[kt][nt] = bbf[:, o : o + N_TILE]

                # Batch 0's A-h1 deints last on the DVE FIFO: that data
                # (lhs for mt 4-7) is only needed at mt group 1 (~14us
                # after the ramp starts), and its transposes issue late —
                # emitting them in k-block order would head-of-line block
                # the ramp-critical deints.
                for kb, (st8, m0, m1, base) in deferred_a:
                    for par in range(2):
                        kt = kb * 2 + par
                        abf = conv_pool.tile(
                            [128, m1 - m0],
                            mybir.dt.bfloat16,
                            name=f"abf_{bi}_{kt}_{m0}",
                            tag=f"abf{kt}_{m0}",
                        )
                        nc.vector.tensor_copy(
                            abf[:],
                            st8[:, base + par : base + 2 * (m1 - m0) : 2],
                        )
                        for mt in range(m0 // M_TILE, m1 // M_TILE):
                            o = mt * M_TILE - m0
                            lhs_ap[kt][mt] = abf[:, o : o + M_TILE]

                # ---- GEMM, accumulating in PSUM over kt. After each mt
                # row's two PSUM-freeing copies (ACT), SYNC issues that
                # row's 512KB store. ----
                def emit_store(mt, ot):
                    st = nc.sync.dma_start(
                        out[bi, mt * M_TILE : (mt + 1) * M_TILE, :], ot[:]
                    )
                    if pending_T:
                        stage_one(*pending_T.pop(0))

                if bi == 0:
                    # Batch 0 is rate-limited by the transpose + deint
                    # stream: iterate kt-outer over groups of 4 mt blocks
                    # (8 PSUM banks) so each arriving k-tile feeds 1.7us
                    # of real PE work and the ramp is gapless. nt-outer
                    # inside each kt so the first 4 matmuls only need
                    # B-half0.
                    for g in range(n_mt // 4):
                        mts = range(4 * g, 4 * g + 4)
                        ps = {
                            (mt, nt): psum_pool.tile(
                                [128, N_TILE],
                                mybir.dt.float32,
                                name=f"ps_{bi}_{mt}_{nt}",
                                tag="ps",
                            )
                            for mt in mts
                            for nt in range(n_nt)
                        }
                        for kt in range(n_kt):
                            for nt in range(n_nt):
                                for mt in mts:
                                    nc.tensor.matmul(
                                        ps[(mt, nt)][:],
                                        lhs_ap[kt][mt],
                                        rhs_ap[kt][nt],
                                        start=(kt == 0),
                                        stop=(kt == n_kt - 1),
                                    )
                        for mt in mts:
                            ot = out_pool.tile(
                                [128, N], mybir.dt.int32, name=f"ot_{bi}_{mt}", tag="ot"
                            )
                            for nt in range(n_nt):
                                nc.scalar.copy(
                                    ot[:, nt * N_TILE : (nt + 1) * N_TILE],
                                    ps[(mt, nt)][:],
                                )
                            emit_store(mt, ot)
                else:
                    # Steady-state batches: mt-outer so the PSUM-freeing
                    # copies and stores spread evenly.
                    for mt in range(n_mt):
                        ps = [
                            psum_pool.tile(
                                [128, N_TILE],
                                mybir.dt.float32,
                                name=f"ps_{bi}_{mt}_{nt}",
                                tag="ps",
                            )
                            for nt in range(n_nt)
                        ]
                        for kt in range(n_kt):
                            for nt in range(n_nt):
                                nc.tensor.matmul(
                                    ps[nt][:],
                                    lhs_ap[kt][mt],
                                    rhs_ap[kt][nt],
                                    start=(kt == 0),
                                    stop=(kt == n_kt - 1),
                                )
                        if bi == BPC - 1 and mt == n_mt - 1:
                            # Final mt row: ACT+DVE copies in parallel into
                            # two half tiles, two 256KB half-stores, so the
                            # kernel tail only waits on the last 256KB.
                            o0 = out_pool.tile(
                                [128, N_TILE],
                                mybir.dt.int32,
                                name="ot_l0",
                                tag="otl0",
                                bufs=1,
                            )
                            o1 = out_pool.tile(
                                [128, N_TILE],
                                mybir.dt.int32,
                                name="ot_l1",
                                tag="otl1",
                                bufs=1,
                            )
                            # Issue the two half-stores from different
                            # engines (ACT + SYNC) so they go out in
                            # parallel; by now no transposes are pending,
                            # so the ACT-ring store cannot stall copies.
                            nc.scalar.copy(o0[:], ps[0][:])
                            nc.vector.tensor_copy(o1[:], ps[1][:])
                            nc.scalar.dma_start(
                                out[bi, mt * M_TILE : (mt + 1) * M_TILE, :N_TILE],
                                o0[:],
                            )
                            nc.sync.dma_start(
                                out[bi, mt * M_TILE : (mt + 1) * M_TILE, N_TILE:],
                                o1[:],
                            )
                        else:
                            ot = out_pool.tile(
                                [128, N], mybir.dt.int32, name=f"ot_{bi}_{mt}", tag="ot"
                            )
                            for nt in range(n_nt):
                                nc.scalar.copy(
                                    ot[:, nt * N_TILE : (nt + 1) * N_TILE], ps[nt][:]
                                )
                            emit_store(mt, ot)
    nc.compile()
    return nc


def _get_nc():
    global _nc_cache
    if _nc_cache is None:
        _nc_cache = build_nc()
    return _nc_cache


def run(a: np.ndarray, b: np.ndarray, trace: bool = False):
    """Run on 8 cores. a/b: [32, 1024, 1024] int8. Returns (out, BassKernelResults)."""
    a = np.ascontiguousarray(a)
    b = np.ascontiguousarray(b)
    a16 = a.view(np.uint16).reshape(B, M, K // 2)
    b16 = b.view(np.uint16).reshape(B, N, K // 2)
    in_maps = [
        {
            "a": a16[c * BPC : (c + 1) * BPC],
            "b": b16[c * BPC : (c + 1) * BPC],
        }
        for c in range(N_CORES)
    ]
    res = run_bass_kernel_spmd(_get_nc(), in_maps, list(range(N_CORES)), trace=trace)
    out = np.concatenate([res.results[c]["out"] for c in range(N_CORES)], axis=0)
    return out, res


def kernel(a: np.ndarray, b: np.ndarray) -> np.ndarray:
    out, _ = run(np.asarray(a), np.asarray(b))
    return out
